# revision 16
# baseline (speedup 1.0000x reference)
"""InvertedReorg (depth-to-space, slice=2) Trainium2 Bass kernel.

Full input x: (32, 256, 64, 64) f32 -> output (32, 64, 128, 128) f32 with
    y[b, c, s1*64 + h, s2*64 + w] = x[b, s1*128 + s2*64 + c, h, w]
i.e. the output image is a 2x2 grid of 64x64 blocks, each block one full
input channel map. Data-parallel over batch: 4 samples per core.

Hybrid two-path schedule (measured fastest of ~10 variants):

* Path A (samples 0-1): direct DRAM->DRAM DMAs on the SWDGE (gpsimd)
  ring. SWDGE's software descriptor generator aggregates the scattered
  256B output rows into 4 KiB packets and sustains ~223 GB/s one-pass.
* Path B (samples 2-3): SBUF round-trip on the two HWDGE rings with
  large descriptors: SP ring loads x[b] into T[p=2c+s1] (16 KiB
  descriptors), DVE does the 256B interleave as a within-partition
  strided copy, ACT ring stores S contiguously (32 KiB descriptors).
  HWDGE is hopeless at direct 256B descriptors (~90 GB/s) but fast at
  large ones (~420 GB/s dual), which is why it gets the SBUF path.

The three rings share the 16 SDMA engines by packet round-robin and
drain concurrently.
"""

import numpy as np

_B, _CH, _H, _W = 32, 256, 64, 64
_NCORES = 8
_BPC = _B // _NCORES  # samples per core
_C = _CH // 4  # output channels
_HW = _H * _W  # 4096
_FD = 2 * _HW  # 8192 free-dim elements per partition

_DIRECT_B = (0, 1)  # samples on the direct SWDGE path
_SBUF_B = (2, 3)  # samples on the SBUF HWDGE pipeline

_cache = {}


def _split_multiwaits(nc, mybir):
    """This walrus build allows one sync-wait command per instruction.
    Tile attaches one wait per dependency, so split the extras into
    same-engine NoOps directly preceding the instruction (the engine
    blocks on each in turn - semantics unchanged)."""
    for f in nc.m.functions:
        for b in f.blocks:
            new_insts = []
            for inst in b.instructions:
                si = inst.sync_info
                if si is not None and len(si.on_wait) > 1:
                    for w in si.on_wait[:-1]:
                        new_insts.append(
                            mybir.InstNoOp(
                                name=f"I-{nc.next_id()}",
                                engine=inst.engine,
                                ins=[],
                                outs=[],
                                sync_info=mybir.SyncInfo(on_wait=[w], on_update=[]),
                            )
                        )
                    inst.sync_info = mybir.SyncInfo(
                        on_wait=[si.on_wait[-1]], on_update=list(si.on_update)
                    )
                new_insts.append(inst)
            b.instructions = new_insts


def _build():
    from concourse import bass, mybir, tile

    nc = bass.Bass()
    x = nc.declare_dram_parameter(
        "x", [_BPC, _CH, _H, _W], mybir.dt.float32, isOutput=False
    )
    y = nc.declare_dram_parameter(
        "y", [_BPC, _C, 2 * _H, 2 * _W], mybir.dt.float32, isOutput=True
    )
    # x viewed as [b, s1, s2, c, (h w)]
    xr = x.rearrange("b (s1 s2 c) h w -> b s1 s2 c (h w)", s1=2, s2=2)
    # y viewed as [b, (c s1), (h w)] -- partition p = 2c + s1 (SBUF path)
    yr = y.rearrange("b c (s hh) w -> b (c s) (hh w)", s=2)
    # y viewed as [b, s1, s2, c, hh, w] (direct path)
    y6 = y.rearrange("b c (s1 hh) (s2 w) -> b s1 s2 c hh w", s1=2, s2=2)

    with tile.TileContext(nc) as tc:
        with (
            tc.tile_pool(name="tin", bufs=2) as pin,
            tc.tile_pool(name="tout", bufs=2) as pout,
        ):
            # Path A: direct one-pass DMAs, issued first so the SWDGE ring
            # starts streaming immediately.
            for b in _DIRECT_B:
                for s1 in range(2):
                    for s2 in range(2):
                        nc.gpsimd.dma_start(
                            out=y6[b, s1, s2],
                            in_=xr[b, s1, s2].rearrange("c (h w) -> c h w", w=_W),
                        )
            # Path B: SBUF pipeline on the HWDGE rings.
            for b in _SBUF_B:
                T = pin.tile([128, _FD], mybir.dt.float32)
                for s1 in range(2):
                    for s2 in range(2):
                        nc.sync.dma_start(
                            out=T[s1::2, s2 * _HW : (s2 + 1) * _HW],
                            in_=xr[b, s1, s2],
                        )
                S = pout.tile([128, _FD], mybir.dt.float32)
                # One 4D-AP copy does the whole interleave, iterated
                # (p, h, s2, w) elementwise -- dst side is contiguous:
                # S[p, h*128 + s2*64 + w] <- T[p, s2*4096 + h*64 + w]
                src = T[:, :].rearrange("p (s2 h w) -> p h s2 w", s2=2, w=_W)
                dst = S.rearrange("p (h s2 w) -> p h s2 w", s2=2, w=_W)
                nc.vector.tensor_copy(out=dst, in_=src)
                # Stores ride the ACT HWDGE ring; loads own the SP ring, so
                # a store waiting on its copy can't head-of-line block the
                # next iteration's loads.
                nc.scalar.dma_start(out=yr[b], in_=S[:, :])
    _split_multiwaits(nc, mybir)
    return nc


def kernel(x: np.ndarray) -> np.ndarray:
    from concourse.bass_utils import run_bass_kernel_spmd

    if "nc" not in _cache:
        _cache["nc"] = _build()
    nc = _cache["nc"]

    x = np.ascontiguousarray(np.asarray(x), dtype=np.float32)
    in_maps = [{"x": x[i * _BPC : (i + 1) * _BPC]} for i in range(_NCORES)]
    res = run_bass_kernel_spmd(nc, in_maps, list(range(_NCORES)))
    return np.concatenate([res.results[i]["y"] for i in range(_NCORES)], axis=0)


# revision 19
# speedup vs baseline: 1.2924x; 1.2924x over previous
"""InvertedReorg (depth-to-space, slice=2) Trainium2 Bass kernel.

Full input x: (32, 256, 64, 64) f32 -> output (32, 64, 128, 128) f32 with
    y[b, c, s1*64 + h, s2*64 + w] = x[b, s1*128 + s2*64 + c, h, w]
i.e. the output image is a 2x2 grid of 64x64 blocks, each block one full
input channel map. Data-parallel over batch: 4 samples per core.

All-direct schedule (measured fastest of ~12 variants): every (b, s1, s2)
channel group is one DRAM->DRAM DMA (source: 64 channel maps x 16 KiB
contiguous; dest: 64x64 rows of 256B at stride 512B), round-robined
across the three DMA rings (SP HWDGE, ACT HWDGE, gpsimd SWDGE). Each
byte crosses HBM exactly once each way with no SBUF round-trip - half
the DMA work of a load/shuffle/store pipeline, no compute, and no
inter-DMA dependencies at all. An SBUF-staged pipeline variant
(95.5 us) and a hybrid (101.8 us) both measured slower than this
(75.3 us); SWDGE aggregates the scattered 256B writes into 4 KiB
packets, and the three rings drain the 16 shared SDMA engines
concurrently.
"""

import numpy as np

_B, _CH, _H, _W = 32, 256, 64, 64
_NCORES = 8
_BPC = _B // _NCORES  # samples per core
_C = _CH // 4  # output channels
_HW = _H * _W  # 4096
_FD = 2 * _HW  # 8192 free-dim elements per partition

_cache = {}


def _split_multiwaits(nc, mybir):
    """This walrus build allows one sync-wait command per instruction.
    Tile attaches one wait per dependency, so split the extras into
    same-engine NoOps directly preceding the instruction (the engine
    blocks on each in turn - semantics unchanged)."""
    for f in nc.m.functions:
        for b in f.blocks:
            new_insts = []
            for inst in b.instructions:
                si = inst.sync_info
                if si is not None and len(si.on_wait) > 1:
                    for w in si.on_wait[:-1]:
                        new_insts.append(
                            mybir.InstNoOp(
                                name=f"I-{nc.next_id()}",
                                engine=inst.engine,
                                ins=[],
                                outs=[],
                                sync_info=mybir.SyncInfo(on_wait=[w], on_update=[]),
                            )
                        )
                    inst.sync_info = mybir.SyncInfo(
                        on_wait=[si.on_wait[-1]], on_update=list(si.on_update)
                    )
                new_insts.append(inst)
            b.instructions = new_insts


def _build():
    from concourse import bass, mybir, tile

    nc = bass.Bass()
    x = nc.declare_dram_parameter(
        "x", [_BPC, _CH, _H, _W], mybir.dt.float32, isOutput=False
    )
    y = nc.declare_dram_parameter(
        "y", [_BPC, _C, 2 * _H, 2 * _W], mybir.dt.float32, isOutput=True
    )
    # x viewed as [b, s1, s2, c, (h w)]
    xr = x.rearrange("b (s1 s2 c) h w -> b s1 s2 c (h w)", s1=2, s2=2)
    # y viewed as [b, (c s1), (h w)] -- partition p = 2c + s1 (SBUF path)
    yr = y.rearrange("b c (s hh) w -> b (c s) (hh w)", s=2)
    # y viewed as [b, s1, s2, c, hh, w] (direct path)
    y6 = y.rearrange("b c (s1 hh) (s2 w) -> b s1 s2 c hh w", s1=2, s2=2)

    engines = [nc.sync, nc.scalar, nc.gpsimd]
    with tile.TileContext(nc) as tc:
        i = 0
        for b in range(_BPC):
            for s1 in range(2):
                for s2 in range(2):
                    src = xr[b, s1, s2].rearrange("c (h w) -> c h w", w=_W)
                    engines[i % 3].dma_start(out=y6[b, s1, s2], in_=src)
                    i += 1
    _split_multiwaits(nc, mybir)
    return nc


def kernel(x: np.ndarray) -> np.ndarray:
    from concourse.bass_utils import run_bass_kernel_spmd

    if "nc" not in _cache:
        _cache["nc"] = _build()
    nc = _cache["nc"]

    x = np.ascontiguousarray(np.asarray(x), dtype=np.float32)
    in_maps = [{"x": x[i * _BPC : (i + 1) * _BPC]} for i in range(_NCORES)]
    res = run_bass_kernel_spmd(nc, in_maps, list(range(_NCORES)))
    return np.concatenate([res.results[i]["y"] for i in range(_NCORES)], axis=0)


# revision 20
# speedup vs baseline: 1.2999x; 1.0058x over previous
"""InvertedReorg (depth-to-space, slice=2) Trainium2 Bass kernel.

Full input x: (32, 256, 64, 64) f32 -> output (32, 64, 128, 128) f32 with
    y[b, c, s1*64 + h, s2*64 + w] = x[b, s1*128 + s2*64 + c, h, w]
i.e. the output image is a 2x2 grid of 64x64 blocks, each block one full
input channel map. Data-parallel over batch: 4 samples per core.

All-direct schedule (measured fastest of ~12 variants): every (b, s1, s2)
channel group is one DRAM->DRAM DMA (source: 64 channel maps x 16 KiB
contiguous; dest: 64x64 rows of 256B at stride 512B), round-robined
across the three DMA rings (SP HWDGE, ACT HWDGE, gpsimd SWDGE). Each
byte crosses HBM exactly once each way with no SBUF round-trip - half
the DMA work of a load/shuffle/store pipeline, no compute, and no
inter-DMA dependencies at all. An SBUF-staged pipeline variant
(95.5 us) and a hybrid (101.8 us) both measured slower than this
(75.3 us); SWDGE aggregates the scattered 256B writes into 4 KiB
packets, and the three rings drain the 16 shared SDMA engines
concurrently.
"""

import numpy as np

_B, _CH, _H, _W = 32, 256, 64, 64
_NCORES = 8
_BPC = _B // _NCORES  # samples per core
_C = _CH // 4  # output channels
_HW = _H * _W  # 4096
_FD = 2 * _HW  # 8192 free-dim elements per partition

_cache = {}


def _split_multiwaits(nc, mybir):
    """This walrus build allows one sync-wait command per instruction.
    Tile attaches one wait per dependency, so split the extras into
    same-engine NoOps directly preceding the instruction (the engine
    blocks on each in turn - semantics unchanged)."""
    for f in nc.m.functions:
        for b in f.blocks:
            new_insts = []
            for inst in b.instructions:
                si = inst.sync_info
                if si is not None and len(si.on_wait) > 1:
                    for w in si.on_wait[:-1]:
                        new_insts.append(
                            mybir.InstNoOp(
                                name=f"I-{nc.next_id()}",
                                engine=inst.engine,
                                ins=[],
                                outs=[],
                                sync_info=mybir.SyncInfo(on_wait=[w], on_update=[]),
                            )
                        )
                    inst.sync_info = mybir.SyncInfo(
                        on_wait=[si.on_wait[-1]], on_update=list(si.on_update)
                    )
                new_insts.append(inst)
            b.instructions = new_insts


def _build():
    from concourse import bass, mybir, tile

    nc = bass.Bass()
    x = nc.declare_dram_parameter(
        "x", [_BPC, _CH, _H, _W], mybir.dt.float32, isOutput=False
    )
    y = nc.declare_dram_parameter(
        "y", [_BPC, _C, 2 * _H, 2 * _W], mybir.dt.float32, isOutput=True
    )
    # x viewed as [b, s1, s2, c, (h w)]
    xr = x.rearrange("b (s1 s2 c) h w -> b s1 s2 c (h w)", s1=2, s2=2)
    # y viewed as [b, (c s1), (h w)] -- partition p = 2c + s1 (SBUF path)
    yr = y.rearrange("b c (s hh) w -> b (c s) (hh w)", s=2)
    # y viewed as [b, s1, s2, c, hh, w] (direct path)
    y6 = y.rearrange("b c (s1 hh) (s2 w) -> b s1 s2 c hh w", s1=2, s2=2)

    # SWDGE (gpsimd) aggregates the 256B rows into 4 KiB packets and runs
    # ~223 GB/s; the HWDGE rings do ~90 GB/s each on this pattern. 6/5/5
    # split, SWDGE issued first since its Q7 startup is the slowest.
    engines = [nc.gpsimd, nc.sync, nc.scalar]
    groups = [
        (b, s1, s2) for b in range(_BPC) for s1 in range(2) for s2 in range(2)
    ]
    assign = [engines[i % 3] for i in range(len(groups))]
    order = sorted(range(len(groups)), key=lambda i: i % 3)
    with tile.TileContext(nc) as tc:
        for i in order:
            b, s1, s2 = groups[i]
            src = xr[b, s1, s2].rearrange("c (h w) -> c h w", w=_W)
            assign[i].dma_start(out=y6[b, s1, s2], in_=src)
    _split_multiwaits(nc, mybir)
    return nc


def kernel(x: np.ndarray) -> np.ndarray:
    from concourse.bass_utils import run_bass_kernel_spmd

    if "nc" not in _cache:
        _cache["nc"] = _build()
    nc = _cache["nc"]

    x = np.ascontiguousarray(np.asarray(x), dtype=np.float32)
    in_maps = [{"x": x[i * _BPC : (i + 1) * _BPC]} for i in range(_NCORES)]
    res = run_bass_kernel_spmd(nc, in_maps, list(range(_NCORES)))
    return np.concatenate([res.results[i]["y"] for i in range(_NCORES)], axis=0)
